# revision 1
# baseline (speedup 1.0000x reference)
"""Dilated multihead attention TRN2 Bass kernel (bf16 datapath).

Problem: B=1, S=4096, E=1024, H=16, d=64.
Configs (seg, dil): (1024,1), (2048,2), (4096,4); r = seg//dil = 1024 for all.
Reference applies the SAME projection Wq to q, k and v, so the projection is
config-independent: compute Xq = q @ Wq.T (etc.) once, and every config's
gathered qs/ks/vs is just a strided row-subset of it.

Sharding: tensor-parallel over heads, 2 heads per core. The Bass program is
identical on all 8 cores; core c receives Wq rows [128c:128c+128) transposed
as data. Each core reads the full (host-pre-transposed) qT/kT/vT.

Key design points vs the f32r baseline:
  - All SBUF-resident attention data is bf16: halves input DMA (24 MiB/core),
    halves DVE copy traffic, and enables the PE fast-weight-load path
    (FWL needs a non-fp32 weight dtype), cutting LDWEIGHTS exposure.
  - The host passes wqT pre-scaled by 1/3 (the config-average factor).
    Scores scale by 1/9 (compensated with exp scale=9*0.125=1.125) and the
    projected V by 1/3, so the per-config output needs no extra 1/3 multiply
    and the denominator row stays unscaled.
  - The two heads' score matmuls are interleaved: head A contracts on array
    rows 0-63, head B on rows 64-127 (tile_position auto-derived from
    base_partition), so consecutive A/B matmuls run concurrently on HW.
  - exp on ScalarE with scale fused (no max subtraction: scores ~ N(0,1),
    |s| small, exp is fp32-safe); V matmul accumulates [d+1, qpos] over
    kpos-tiles with the denominator in row 64 (ones column in the lhsT).
  - Normalize: reciprocal of the denominator row, partition-broadcast on
    GpSimd, multiply(+add) into acc [128, S] f32 (head h at partitions 64h).

key_padding_mask is all zeros by construction (spec fill=zeros) and is
therefore not applied on device.
"""

import os

# The axon-tunneled NeuronCores occasionally come up wedged from a prior
# session (NRT_EXEC_UNIT_UNRECOVERABLE on first use); requesting a core
# reset at runtime-init clears it and is a no-op on a healthy device.
os.environ.setdefault("NEURON_RT_RESET_CORES", "1")

import numpy as np

import concourse.bass as bass
import concourse.bacc as bacc
import concourse.tile as tile
from concourse import mybir
from concourse.bass_utils import run_bass_kernel_spmd

S = 4096
E = 1024
HD = 128  # head dims per core (2 heads x 64)
NCORES = 8
CHUNK = 512  # positions per projection chunk
NCHUNK = S // CHUNK
CONFIGS = [(1024, 1), (2048, 2), (4096, 4)]


def _units_ready_after_chunk():
    """Map chunk index -> list of (cfg_idx, seg_idx) whose positions are
    fully projected once that chunk is done."""
    ready = {c: [] for c in range(NCHUNK)}
    for ci, (seg, dil) in enumerate(CONFIGS):
        for j in range(S // seg):
            last_pos = (j + 1) * seg - 1
            ready[last_pos // CHUNK].append((ci, j))
    return ready


def build_bass(loop_n=None, stage_level=4):
    """loop_n: if set, wrap the whole body in an on-device For_i repeat
    loop (timing mode: marginal wall time per extra iteration = HW exec
    time, independent of host dispatch overhead)."""
    f32 = mybir.dt.float32
    bf16 = mybir.dt.bfloat16
    nc = bacc.Bacc("TRN2", target_bir_lowering=False, debug=False,
                   num_devices=NCORES)
    qT = nc.declare_dram_parameter("qT", [E, S], bf16, isOutput=False)
    kT = nc.declare_dram_parameter("kT", [E, S], bf16, isOutput=False)
    vT = nc.declare_dram_parameter("vT", [E, S], bf16, isOutput=False)
    wqT = nc.declare_dram_parameter("wqT", [E, HD], bf16, isOutput=False)
    ident = nc.declare_dram_parameter("ident", [128, 128], bf16,
                                      isOutput=False)
    outT = nc.declare_dram_parameter("outT", [HD, S], f32, isOutput=True)

    ET = E // 128  # 8 E-tiles

    with tile.TileContext(nc) as tc:
        # ---- persistent SBUF tensors ----
        _frees = []

        def ptile(shape, name, dt=f32):
            t, free = tc.tile(shape, dt, name=name)
            _frees.append(free)
            return t

        wq_sb = ptile([128, ET, HD], "wq_sb", bf16)
        # X buffers are ping-ponged across For_i iterations so iteration
        # i+1's DMA+projection can overlap iteration i's attention tail
        # (otherwise the write-after-read dependency on XqT serializes
        # consecutive iterations).
        nv_tiles = [S // 128 // dil for (seg, dil) in CONFIGS]  # 32,16,8
        XqT, XkT, Xv = [], [], []
        for pp in range(2):
            XqT.append(ptile([HD, S], f"XqT{pp}", bf16))
            XkT.append(ptile([HD, S], f"XkT{pp}", bf16))
            # Xv per config: gathered [kpos, (64|1)*2] tiles, 130 cols/tile
            Xv.append([ptile([128, n * 130], f"Xv{pp}_{i}", bf16)
                       for i, n in enumerate(nv_tiles)])
            for xv in Xv[pp]:
                nc.vector.memset(xv[:, 64::65], 1.0)  # ones cols (denom)
        # per-head accumulators (walrus requires TensorTensor operands to
        # share a start partition, so both heads accumulate at base 0)
        acc = [ptile([64, S], "acc0"), ptile([64, S], "acc1")]
        id_sb = ptile([128, 128], "id_sb", bf16)

        # ---- pools ----
        import contextlib
        ctx = contextlib.ExitStack()
        with ctx:
            stage = ctx.enter_context(tc.tile_pool(name="stage", bufs=3))
            xvt_pool = ctx.enter_context(tc.tile_pool(name="xvt", bufs=3))
            wt_pool = ctx.enter_context(tc.tile_pool(name="wt", bufs=31))
            rc_pool = ctx.enter_context(tc.tile_pool(name="rc", bufs=3))
            bc_pool = ctx.enter_context(tc.tile_pool(name="bc", bufs=3))
            tmp_pool = ctx.enter_context(tc.tile_pool(name="tmp", bufs=2))
            sg_pool = ctx.enter_context(tc.tile_pool(name="sg", bufs=6))
            ps_sc = ctx.enter_context(
                tc.tile_pool(name="ps_sc", bufs=2, space="PSUM"))
            ps_v = ctx.enter_context(
                tc.tile_pool(name="ps_v", bufs=2, space="PSUM"))
            ps_wk = ctx.enter_context(
                tc.tile_pool(name="ps_wk", bufs=2, space="PSUM"))

            # load wqT: [E, HD] -> [128, ET, HD]
            nc.sync.dma_start(
                wq_sb[:], wqT.rearrange("(a p) m -> p a m", p=128))
            nc.sync.dma_start(id_sb[:], ident[:])

            ready = _units_ready_after_chunk()

            def proj_chunk(pp, c):
                lo = c * CHUNK
                xs = []
                for i, (src, nm) in enumerate(
                        ((qT, "qc"), (kT, "kc"), (vT, "vc"))):
                    t = stage.tile([128, ET, CHUNK], bf16, name=nm,
                                   tag="stage")
                    # balance DMA issue across the two DGE rings
                    on_pool = (i == 1) or (i == 2 and c % 2 == 1)
                    eng = nc.gpsimd if on_pool else nc.sync
                    eng.dma_start(
                        t[:],
                        src.rearrange("(a p) n -> p a n", p=128)
                           [:, :, lo:lo + CHUNK])
                    xs.append(t)
                qc, kc, vc = xs
                # q,k projections -> XqT/XkT transposed (bf16)
                for src, dst in ((qc, XqT[pp]), (kc, XkT[pp])):
                    ps = ps_wk.tile([128, CHUNK], f32, name="ps_proj",
                                    tag="wk")
                    for e in range(ET):
                        nc.tensor.matmul(ps[:], wq_sb[:, e, :], src[:, e, :],
                                         start=(e == 0), stop=(e == ET - 1))
                    nc.vector.tensor_copy(dst[:, lo:lo + CHUNK], ps[:])
                # v projection: one transposed XvT per chunk; each config's
                # gathered Xv tiles come from strided column subsets of it
                # via PE transposes.
                ps = ps_wk.tile([128, CHUNK], f32, name="ps_vt", tag="wk")
                for e in range(ET):
                    nc.tensor.matmul(ps[:], wq_sb[:, e, :], vc[:, e, :],
                                     start=(e == 0), stop=(e == ET - 1))
                xvt = xvt_pool.tile([128, CHUNK], bf16, name="xvt", tag="xvt")
                nc.vector.tensor_copy(xvt[:], ps[:])
                for ci, (seg, dil) in enumerate(CONFIGS):
                    npt = CHUNK // dil // 128  # transposes: 4,2,1
                    for t in range(npt):
                        g = c * npt + t  # global gathered tile index
                        pt_ = ps_wk.tile([128, 128], bf16, name="ps_tr",
                                         tag="wk")
                        sl = slice(t * 128 * dil, (t + 1) * 128 * dil, dil)
                        nc.tensor.transpose(pt_[:], xvt[:, sl], id_sb[:])
                        dst = Xv[pp][ci][:, 130 * g:130 * (g + 1)] \
                            .rearrange("p (a b) -> p a b", b=65)[:, :, 0:64]
                        nc.vector.tensor_copy(
                            dst, pt_[:].rearrange("p (a b) -> p a b", b=64))

            def attention_scores(pp, ci, j):
                """Emit the scores+exp phase (ACT-bound) for one unit;
                returns the per-(head, kt) exp'd weight tiles."""
                seg, dil = CONFIGS[ci]
                r = seg // dil  # 1024 gathered positions
                assert r == 1024
                wts = [[None] * 8, [None] * 8]  # per head
                for kt in range(8):
                    ksl = slice(j * seg + kt * 128 * dil,
                                j * seg + (kt + 1) * 128 * dil, dil)
                    pss = [ps_sc.tile([128, r], f32, name=f"ps_s{h}",
                                      tag="sc") for h in (0, 1)]
                    for half in (0, 1):
                        q2 = slice(j * seg + half * 512 * dil,
                                   j * seg + (half + 1) * 512 * dil, dil)
                        for h in (0, 1):  # interleave A/B for row packing
                            hsl = slice(64 * h, 64 * h + 64)
                            nc.tensor.matmul(
                                pss[h][:, half * 512:(half + 1) * 512],
                                XkT[pp][hsl, ksl], XqT[pp][hsl, q2])
                    if stage_level < 2:
                        continue
                    for h in (0, 1):
                        wt = wt_pool.tile([128, r], bf16, name="wt", tag="wt")
                        nc.scalar.activation(
                            wt[:], pss[h][:],
                            mybir.ActivationFunctionType.Exp, scale=1.125)
                        wts[h][kt] = wt
                return wts

            def attention_v(pp, ci, j, wts):
                """Dense V bursts per head (PE-bound); psum accumulators
                are staged to SBUF with one cheap copy each (freeing the
                psum slots fast) and the normalize chains trail lazily on
                DVE/GpSimd, off the PE critical path."""
                seg, dil = CONFIGS[ci]
                gbase = j * seg // dil // 128  # Xv tile base (8 per unit)
                if stage_level < 3:
                    return
                staged = []
                for h in (0, 1):
                    ov = [None, None]
                    for kc in range(8):
                        g = gbase + kc
                        lhs = Xv[pp][ci][:, 130 * g + 65 * h:
                                         130 * g + 65 * h + 65]
                        for qt in (0, 1):
                            if kc == 0:
                                ov[qt] = ps_v.tile([65, 512], f32,
                                                   name="ov", tag="ov")
                            nc.tensor.matmul(
                                ov[qt][:], lhs,
                                wts[h][kc][:, qt * 512:(qt + 1) * 512],
                                start=(kc == 0), stop=(kc == 7))
                    if stage_level < 4:
                        continue
                    sgs = []
                    for qt in (0, 1):
                        sg = sg_pool.tile([65, 512], bf16, name="sg",
                                          tag="sg")
                        nc.vector.tensor_copy(sg[:], ov[qt][:])
                        sgs.append(sg)
                    staged.append(sgs)
                if stage_level < 4:
                    return
                for h in (0, 1):
                    for qt in (0, 1):
                        o = staged[h][qt]
                        rc = rc_pool.tile([1, 512], f32, name="rc",
                                          tag="rc")
                        nc.vector.reciprocal(rc[:], o[64:65, :])
                        bc = bc_pool.tile([64, 512], f32, name="bc",
                                          tag="bc")
                        nc.gpsimd.partition_broadcast(bc[:], rc[:])
                        a0 = j * seg + qt * 512 * dil
                        tgt = acc[h][:, a0:a0 + 512 * dil:dil]
                        if ci == 0:
                            nc.vector.tensor_mul(tgt, o[0:64, :], bc[:])
                        else:
                            tmp = tmp_pool.tile([64, 512], f32, name="tmp",
                                                tag="tmp")
                            nc.vector.tensor_mul(tmp[:], o[0:64, :], bc[:])
                            nc.vector.tensor_add(tgt, tgt, tmp[:])

            def body(pp):
                # Software-pipeline units: each unit's V phase is emitted
                # AFTER the next unit's scores phase, so ACT (exp of the
                # next unit) stays fed while PE runs the V bursts. The
                # scheduler still hoists V ahead wherever scores are
                # blocked on projection DMA.
                pending = None  # (ci, j, wts)
                for c in range(NCHUNK):
                    proj_chunk(pp, c)
                    if stage_level >= 1:
                        for (ci, j) in ready[c]:
                            wts = attention_scores(pp, ci, j)
                            if pending is not None:
                                attention_v(pp, *pending)
                            pending = (ci, j, wts)
                if pending is not None:
                    attention_v(pp, *pending)

                if stage_level >= 4:
                    nc.sync.dma_start(outT[0:64, :], acc[0][:])
                    nc.gpsimd.dma_start(outT[64:128, :], acc[1][:])

            if loop_n is None:
                body(0)
            else:
                assert loop_n % 2 == 0, "loop_n must be even (ping-pong)"
                with tc.For_i(0, loop_n // 2, 1):
                    body(0)
                    body(1)

        for f in reversed(_frees):
            f()

    nc.compile()
    return nc


_CACHED = {}


def make_in_maps(query, key, value, Wq):
    """Host-side input prep: bf16 transposes + the Wq/3 scaling trick."""
    import ml_dtypes
    bf = ml_dtypes.bfloat16
    qT = np.ascontiguousarray(query[0].T).astype(bf)
    kT = np.ascontiguousarray(key[0].T).astype(bf)
    vT = np.ascontiguousarray(value[0].T).astype(bf)
    ident = np.eye(128, dtype=np.float32).astype(bf)
    in_maps = []
    for c in range(NCORES):
        wqTc = np.ascontiguousarray(
            (Wq[HD * c:HD * (c + 1), :] / 3.0).T).astype(bf)
        in_maps.append({"qT": qT, "kT": kT, "vT": vT, "wqT": wqTc,
                        "ident": ident})
    return in_maps


def kernel(query, key, value, key_padding_mask, Wq):
    query = np.asarray(query, dtype=np.float32)
    key = np.asarray(key, dtype=np.float32)
    value = np.asarray(value, dtype=np.float32)
    Wq = np.asarray(Wq, dtype=np.float32)
    assert query.shape == (1, S, E), query.shape

    if "nc" not in _CACHED:
        _CACHED["nc"] = build_bass()
    nc = _CACHED["nc"]

    in_maps = make_in_maps(query, key, value, Wq)
    res = run_bass_kernel_spmd(nc, in_maps, list(range(NCORES)))
    outT = np.concatenate([res.results[c]["outT"] for c in range(NCORES)],
                          axis=0)  # [E, S]
    return np.ascontiguousarray(outT.T)[None].astype(np.float32)



# revision 29
# speedup vs baseline: 1.3179x; 1.3179x over previous
"""Dilated multihead attention TRN2 Bass kernel (bf16 datapath).

Problem: B=1, S=4096, E=1024, H=16, d=64.
Configs (seg, dil): (1024,1), (2048,2), (4096,4); r = seg//dil = 1024 for all.
Reference applies the SAME projection Wq to q, k and v, so the projection is
config-independent: compute Xq = q @ Wq.T (etc.) once, and every config's
gathered qs/ks/vs is just a strided row-subset of it.

Sharding: tensor-parallel over heads, 2 heads per core. The Bass program is
identical on all 8 cores; core c receives Wq rows [128c:128c+128) transposed
as data. Each core reads the full (host-pre-transposed) qT/kT/vT.

Engine-budget analysis (per core): exp of all scores = 2 heads x 7 units x
1M = 14.7M elements on ScalarE at ~1 elem/cycle/lane @1.2GHz ~= 112us; PE
matmul work (proj 98k + scores 57k + V 115k cycles @2.4GHz) ~= 117us. Both
engines must stay saturated and overlapped; everything else (DVE copies,
DMA 24MiB in) hides underneath.

Design points:
  - bf16 SBUF datapath; host passes wqT pre-scaled by 1/3 (the config-average
    factor): scores come out scaled 1/9 (exp scale=9*0.125), projected V by
    1/3, so per-config outputs need no extra multiply.
  - Scores matmuls row-packed: head A contracts on PE rows 0-63, head B on
    rows 64-127 (tile_position auto-derived) -> concurrent on HW.
  - V matmul lhsT = gathered Xv [128 kpos, 65] with a ones column: PSUM row
    64 accumulates the softmax denominator for free during the V streams.
  - Softmax DIVISION IS DONE ON HOST: the device DMAs out raw [65, 512]
    (d + denominator, qpos) tiles; the host divides and scatter-adds the
    three config grids. This removes the reciprocal/broadcast/multiply/add
    chains (~40us of DVE+GpSimd) and their pipeline dependencies entirely.
  - The v-projection path (vc DMA, v proj, PE gather-transposes, Xv copies)
    is emitted AFTER each chunk's attention work: it is off the critical
    scores->exp path and fills PE/DVE gaps.

key_padding_mask is all zeros by construction (spec fill=zeros) and is
therefore not applied on device.
"""

import os

# The axon-tunneled NeuronCores occasionally come up wedged from a prior
# session (NRT_EXEC_UNIT_UNRECOVERABLE on first use); requesting a core
# reset at runtime-init clears it and is a no-op on a healthy device.
os.environ.setdefault("NEURON_RT_RESET_CORES", "1")

import numpy as np

import concourse.bass as bass
import concourse.bacc as bacc
import concourse.tile as tile
from concourse import mybir
from concourse.bass_utils import run_bass_kernel_spmd

S = 4096
E = 1024
HD = 128  # head dims per core (2 heads x 64)
NCORES = 8
CHUNK = 512  # positions per projection chunk
NCHUNK = S // CHUNK
CONFIGS = [(1024, 1), (2048, 2), (4096, 4)]

# Units (ci, j) ordered by the chunk after which they are fully projected.
UNITS = [(0, 0), (0, 1), (1, 0), (0, 2), (0, 3), (1, 1), (2, 0)]
NUNITS = len(UNITS)
UID = {u: i for i, u in enumerate(UNITS)}


def _units_ready_after_chunk():
    ready = {c: [] for c in range(NCHUNK)}
    for ci, (seg, dil) in enumerate(CONFIGS):
        for j in range(S // seg):
            last_pos = (j + 1) * seg - 1
            ready[last_pos // CHUNK].append((ci, j))
    return ready


def build_bass(loop_n=None, stage_level=4):
    """loop_n: if set, wrap the whole body in an on-device For_i repeat
    loop (timing mode: marginal wall time per extra iteration = HW exec
    time, independent of host dispatch overhead)."""
    f32 = mybir.dt.float32
    bf16 = mybir.dt.bfloat16
    nc = bacc.Bacc("TRN2", target_bir_lowering=False, debug=False,
                   num_devices=NCORES)
    qT = nc.declare_dram_parameter("qT", [E, S], bf16, isOutput=False)
    kT = nc.declare_dram_parameter("kT", [E, S], bf16, isOutput=False)
    vT = nc.declare_dram_parameter("vT", [E, S], bf16, isOutput=False)
    wqT = nc.declare_dram_parameter("wqT", [E, HD], bf16, isOutput=False)
    ident = nc.declare_dram_parameter("ident", [128, 128], bf16,
                                      isOutput=False)
    # raw output tiles: [d+den, unit*2048 + h*1024 + qt*512 + qcol]
    o_out = nc.declare_dram_parameter("o_out", [65, NUNITS * 2048], f32,
                                      isOutput=True)

    ET = E // 128  # 8 E-tiles

    with tile.TileContext(nc) as tc:
        # ---- persistent SBUF tensors ----
        _frees = []

        def ptile(shape, name, dt=f32):
            t, free = tc.tile(shape, dt, name=name)
            _frees.append(free)
            return t

        wq_sb = ptile([128, ET, HD], "wq_sb", bf16)
        # X buffers ping-ponged across For_i iterations so iteration i+1's
        # DMA+projection can overlap iteration i's attention tail.
        nv_tiles = [S // 128 // dil for (seg, dil) in CONFIGS]  # 32,16,8
        XqT, XkT, Xv = [], [], []
        for pp in range(2):
            XqT.append(ptile([HD, S], f"XqT{pp}", bf16))
            XkT.append(ptile([HD, S], f"XkT{pp}", bf16))
            # Xv per config: gathered [kpos, (64|1)*2] tiles, 130 cols/tile
            Xv.append([ptile([128, n * 130], f"Xv{pp}_{i}", bf16)
                       for i, n in enumerate(nv_tiles)])
            for xv in Xv[pp]:
                nc.vector.memset(xv[:, 64::65], 1.0)  # ones cols (denom)
        id_sb = ptile([128, 128], "id_sb", bf16)

        # ---- pools ----
        import contextlib
        ctx = contextlib.ExitStack()
        with ctx:
            stage = ctx.enter_context(tc.tile_pool(name="stage", bufs=10))
            xvt_pool = ctx.enter_context(tc.tile_pool(name="xvt", bufs=3))
            wt_pool = ctx.enter_context(tc.tile_pool(name="wt", bufs=24))
            sg_pool = ctx.enter_context(tc.tile_pool(name="sg", bufs=4))
            # PSUM budget (8 banks): scores 2x[128,1024]f32 (4 banks),
            # V accumulators 2x[65,512] (2 banks), proj/transpose 2 banks.
            ps_sc = ctx.enter_context(
                tc.tile_pool(name="ps_sc", bufs=2, space="PSUM"))
            ps_v = ctx.enter_context(
                tc.tile_pool(name="ps_v", bufs=2, space="PSUM"))
            ps_wk = ctx.enter_context(
                tc.tile_pool(name="ps_wk", bufs=2, space="PSUM"))

            # load wqT: [E, HD] -> [128, ET, HD]  (scalar ring: keeps the
            # sync ring free for the first q-chunk DMA)
            nc.scalar.dma_start(
                wq_sb[:], wqT.rearrange("(a p) m -> p a m", p=128))
            nc.scalar.dma_start(id_sb[:], ident[:])

            ready = _units_ready_after_chunk()

            import contextlib as _ctxlib

            @_ctxlib.contextmanager
            def lowprio(off=100000):
                """Emit instructions with scheduler priority pushed towards
                the end of the program (deps still enforce correctness)."""
                tc.cur_priority += off
                try:
                    yield
                finally:
                    tc.cur_priority -= off

            def dma_chunk(src, nm, c, eng):
                lo = c * CHUNK
                t = stage.tile([128, ET, CHUNK], bf16, name=nm, tag="stage")
                eng.dma_start(
                    t[:],
                    src.rearrange("(a p) n -> p a n", p=128)
                       [:, :, lo:lo + CHUNK])
                return t

            def proj_x(pp, c, st, dst):
                """Project one staged chunk tensor into dst[:, chunk]."""
                lo = c * CHUNK
                ps = ps_wk.tile([128, CHUNK], f32, name="ps_proj", tag="wk")
                for e in range(ET):
                    nc.tensor.matmul(ps[:], wq_sb[:, e, :], st[:, e, :],
                                     start=(e == 0), stop=(e == ET - 1))
                nc.vector.tensor_copy(dst[:, lo:lo + CHUNK], ps[:])

            def proj_qk(pp, c):
                """q/k chunk DMA + projection: the critical path feeding
                scores->exp; emitted first within each chunk block."""
                qc = dma_chunk(qT, "qc", c, nc.sync)
                kc = dma_chunk(kT, "kc", c, nc.gpsimd)
                proj_x(pp, c, qc, XqT[pp])
                proj_x(pp, c, kc, XkT[pp])

            def proj_v(pp, c):
                """v chunk DMA + projection + gather-transposes; off the
                critical path (consumed by V matmuls ~15us later); its DMA
                is emitted after the prefetched q/k loads on the rings."""
                lo = c * CHUNK
                vc = stage.tile([128, ET, CHUNK], bf16, name="vc",
                                tag="stage")
                eng = nc.gpsimd if c % 2 == 1 else nc.sync
                eng.dma_start(
                    vc[:],
                    vT.rearrange("(a p) n -> p a n", p=128)
                      [:, :, lo:lo + CHUNK])
                ps = ps_wk.tile([128, CHUNK], f32, name="ps_vt", tag="wk")
                for e in range(ET):
                    nc.tensor.matmul(ps[:], wq_sb[:, e, :], vc[:, e, :],
                                     start=(e == 0), stop=(e == ET - 1))
                xvt = xvt_pool.tile([128, CHUNK], bf16, name="xvt", tag="xvt")
                nc.vector.tensor_copy(xvt[:], ps[:])
                for ci, (seg, dil) in enumerate(CONFIGS):
                    npt = CHUNK // dil // 128  # transposes: 4,2,1
                    for t in range(npt):
                        g = c * npt + t  # global gathered tile index
                        pt_ = ps_wk.tile([128, 128], bf16, name="ps_tr",
                                         tag="wk")
                        sl = slice(t * 128 * dil, (t + 1) * 128 * dil, dil)
                        nc.tensor.transpose(pt_[:], xvt[:, sl], id_sb[:])
                        dst = Xv[pp][ci][:, 130 * g:130 * (g + 1)] \
                            .rearrange("p (a b) -> p a b", b=65)[:, :, 0:64]
                        nc.vector.tensor_copy(
                            dst, pt_[:].rearrange("p (a b) -> p a b", b=64))

            def attention_scores(pp, ci, j, kts=range(8), wts=None,
                                 cb=None):
                """Scores + exp for one unit; returns per-(head, kt) exp'd
                weight tiles. Head A/B matmuls interleave on PE row halves.
                cb(kt) (if given) emits interleaved work after each kt
                bundle — used to pace the previous unit's V matmuls so they
                never stuff the PE exec queue ahead of scores."""
                seg, dil = CONFIGS[ci]
                r = seg // dil
                assert r == 1024
                if wts is None:
                    wts = [[None] * 8, [None] * 8]  # per head
                for kt in kts:
                    ksl = slice(j * seg + kt * 128 * dil,
                                j * seg + (kt + 1) * 128 * dil, dil)
                    pss = [ps_sc.tile([128, r], f32, name=f"ps_s{h}",
                                      tag="sc") for h in (0, 1)]
                    for half in (0, 1):
                        q2 = slice(j * seg + half * 512 * dil,
                                   j * seg + (half + 1) * 512 * dil, dil)
                        for h in (0, 1):  # interleave A/B for row packing
                            hsl = slice(64 * h, 64 * h + 64)
                            nc.tensor.matmul(
                                pss[h][:, half * 512:(half + 1) * 512],
                                XkT[pp][hsl, ksl], XqT[pp][hsl, q2])
                    if stage_level < 2:
                        continue
                    for h in (0, 1):
                        wt = wt_pool.tile([128, r], bf16, name="wt", tag="wt")
                        nc.scalar.activation(
                            wt[:], pss[h][:],
                            mybir.ActivationFunctionType.Exp, scale=1.125)
                        wts[h][kt] = wt
                    if cb is not None:
                        cb(kt)
                return wts

            def v_piece(pp, uid, ci, j, wts, ov, piece):
                """One of 8 V-burst pieces for a unit: piece p covers
                head p//4, q-half (p//2)%2, kc quad (p%2)*4..+3 — exactly
                one [65, 512] psum accumulator (d rows + denominator row
                64) is live at a time; after its kc7 matmul it is staged
                to SBUF and DMA'd out raw — softmax division happens on
                the host."""
                if stage_level < 3:
                    return
                seg, dil = CONFIGS[ci]
                gbase = j * seg // dil // 128  # Xv tile base (8 per unit)
                h, qt, kh = piece // 4, (piece // 2) % 2, piece % 2
                for kc in range(4 * kh, 4 * kh + 4):
                    lhs = Xv[pp][ci][:, 130 * (gbase + kc) + 65 * h:
                                     130 * (gbase + kc) + 65 * h + 65]
                    if kc == 0:
                        ov[h][qt] = ps_v.tile([65, 512], f32,
                                              name="ov", tag="ov")
                    nc.tensor.matmul(
                        ov[h][qt][:], lhs,
                        wts[h][kc][:, qt * 512:(qt + 1) * 512],
                        start=(kc == 0), stop=(kc == 7))
                if kh == 1 and stage_level >= 4:
                    sg = sg_pool.tile([65, 512], f32, name="sg", tag="sg")
                    nc.vector.tensor_copy(sg[:], ov[h][qt][:])
                    base = uid * 2048 + h * 1024 + qt * 512
                    eng = nc.sync if (h + qt) % 2 == 0 else nc.gpsimd
                    eng.dma_start(o_out[:, base:base + 512], sg[:])

            def attention_v(pp, uid, ci, j, wts):
                """All 8 V pieces back to back (used for the final unit;
                its matmuls still dispatch early, piece by piece, as the
                exp tiles they need are produced)."""
                ov = [[None, None], [None, None]]
                for piece in range(8):
                    v_piece(pp, uid, ci, j, wts, ov, piece)

            def mk_vcb(pp, pv):
                if pv is None:
                    return None
                ov = [[None, None], [None, None]]

                def cb(kt):
                    v_piece(pp, *pv, ov, kt)
                return cb

            def body(pp):
                # Emission (= scheduler priority) order per chunk:
                # q/k DMA+proj, then all newly-ready units' scores+exp,
                # then the v-projection path, then pending V bursts (the
                # V deps land after proj_v so program-order deps hold;
                # the list scheduler hoists ready V matmuls into PE gaps
                # while ACT paces through the exp queue).
                pending = None  # (uid, ci, j, wts)
                # Fast lead-in: q/k DMAs for chunks 0-3 go out up front on
                # 3 rings (DMA queues drain in emission order; prefetching
                # keeps scores-feeding loads ahead of vc/o_out traffic),
                # and unit (0,0)'s scores kt0-3 (which need only q0/q1/k0)
                # are emitted before the k1 projection so ACT starts early.
                qk = {}
                qk[0] = (dma_chunk(qT, "qc", 0, nc.sync),
                         dma_chunk(kT, "kc", 0, nc.gpsimd))
                qk[1] = (dma_chunk(qT, "qc", 1, nc.scalar),
                         dma_chunk(kT, "kc", 1, nc.sync))
                qk[2] = (dma_chunk(qT, "qc", 2, nc.gpsimd),
                         dma_chunk(kT, "kc", 2, nc.sync))
                qk[3] = (dma_chunk(qT, "qc", 3, nc.gpsimd),
                         dma_chunk(kT, "kc", 3, nc.sync))
                proj_x(pp, 0, qk[0][0], XqT[pp])
                proj_x(pp, 0, qk[0][1], XkT[pp])
                proj_x(pp, 1, qk[1][0], XqT[pp])
                if stage_level >= 1:
                    wts00 = attention_scores(pp, 0, 0, kts=range(0, 4))
                proj_x(pp, 1, qk[1][1], XkT[pp])
                if stage_level >= 1:
                    attention_scores(pp, 0, 0, kts=range(4, 8), wts=wts00)
                    pending = (UID[(0, 0)], 0, 0, wts00)
                proj_v(pp, 0)
                proj_v(pp, 1)
                pend = [(pending, 1)] if pending is not None else []
                for c in range(2, NCHUNK):
                    if c + 2 < NCHUNK:  # depth-2 q/k prefetch
                        qk[c + 2] = (dma_chunk(qT, "qc", c + 2, nc.gpsimd),
                                     dma_chunk(kT, "kc", c + 2, nc.sync))
                    proj_x(pp, c, qk[c][0], XqT[pp])
                    proj_x(pp, c, qk[c][1], XkT[pp])
                    del qk[c]
                    if stage_level >= 1:
                        for (ci, j) in ready[c]:
                            # interleave the oldest pending unit's V pieces
                            # into this unit's scores — legal only if its
                            # v-projection (chunk < c) is already emitted
                            cb = None
                            if pend and pend[0][1] < c:
                                cb = mk_vcb(pp, pend.pop(0)[0])
                            wts = attention_scores(pp, ci, j, cb=cb)
                            pend.append(((UID[(ci, j)], ci, j, wts), c))
                    proj_v(pp, c)
                for args, _ in pend:
                    attention_v(pp, *args)

            if loop_n is None:
                body(0)
            else:
                assert loop_n % 2 == 0, "loop_n must be even (ping-pong)"
                with tc.For_i(0, loop_n // 2, 1):
                    body(0)
                    body(1)

        for f in reversed(_frees):
            f()

    nc.compile()
    return nc


_CACHED = {}


def make_in_maps(query, key, value, Wq):
    """Host-side input prep: bf16 transposes + the Wq/3 scaling trick."""
    import ml_dtypes
    bf = ml_dtypes.bfloat16
    qT = np.ascontiguousarray(query[0].T).astype(bf)
    kT = np.ascontiguousarray(key[0].T).astype(bf)
    vT = np.ascontiguousarray(value[0].T).astype(bf)
    ident = np.eye(128, dtype=np.float32).astype(bf)
    in_maps = []
    for c in range(NCORES):
        wqTc = np.ascontiguousarray(
            (Wq[HD * c:HD * (c + 1), :] / 3.0).T).astype(bf)
        in_maps.append({"qT": qT, "kT": kT, "vT": vT, "wqT": wqTc,
                        "ident": ident})
    return in_maps


def assemble(o_outs):
    """Host-side softmax division + strided scatter-sum of the 3 config
    grids. o_outs: per-core [65, NUNITS*2048] f32 raw tiles."""
    out = np.zeros((S, E), np.float32)
    for c, o in enumerate(o_outs):
        t = np.asarray(o, np.float32).reshape(65, NUNITS, 2, 2, 512)
        val = t[0:64] / t[64:65]          # [64, u, h, qt, col]
        for ci, (seg, dil) in enumerate(CONFIGS):
            n_seg = S // seg
            uids = [UID[(ci, j)] for j in range(n_seg)]
            v = val[:, uids]              # [64, n_seg, h, qt, col]
            # gathered pos = qt*512+col ; head dims = h*64+d
            g = v.transpose(1, 3, 4, 2, 0).reshape(n_seg, 1024, HD)
            out.reshape(n_seg, seg, E)[:, ::dil, HD * c:HD * (c + 1)] += g
    return out


def kernel(query, key, value, key_padding_mask, Wq):
    query = np.asarray(query, dtype=np.float32)
    key = np.asarray(key, dtype=np.float32)
    value = np.asarray(value, dtype=np.float32)
    Wq = np.asarray(Wq, dtype=np.float32)
    assert query.shape == (1, S, E), query.shape

    if "nc" not in _CACHED:
        _CACHED["nc"] = build_bass()
    nc = _CACHED["nc"]

    in_maps = make_in_maps(query, key, value, Wq)
    res = run_bass_kernel_spmd(nc, in_maps, list(range(NCORES)))
    out = assemble([res.results[c]["o_out"] for c in range(NCORES)])
    return out[None].astype(np.float32)


# revision 31
# speedup vs baseline: 1.3304x; 1.0095x over previous
"""Dilated multihead attention TRN2 Bass kernel (bf16 datapath).

Problem: B=1, S=4096, E=1024, H=16, d=64.
Configs (seg, dil): (1024,1), (2048,2), (4096,4); r = seg//dil = 1024 for all.
Reference applies the SAME projection Wq to q, k and v, so the projection is
config-independent: compute Xq = q @ Wq.T (etc.) once, and every config's
gathered qs/ks/vs is just a strided row-subset of it.

Sharding: tensor-parallel over heads, 2 heads per core. The Bass program is
identical on all 8 cores; core c receives Wq rows [128c:128c+128) transposed
as data. Each core reads the full (host-pre-transposed) qT/kT/vT.

Engine-budget analysis (per core): exp of all scores = 2 heads x 7 units x
1M = 14.7M elements on ScalarE at ~1 elem/cycle/lane @1.2GHz ~= 112us; PE
matmul work (proj 98k + scores 57k + V 115k cycles @2.4GHz) ~= 117us. Both
engines must stay saturated and overlapped; everything else (DVE copies,
DMA 24MiB in) hides underneath.

Design points:
  - bf16 SBUF datapath; host passes wqT pre-scaled by 1/3 (the config-average
    factor): scores come out scaled 1/9 (exp scale=9*0.125), projected V by
    1/3, so per-config outputs need no extra multiply.
  - Scores matmuls row-packed: head A contracts on PE rows 0-63, head B on
    rows 64-127 (tile_position auto-derived) -> concurrent on HW.
  - V matmul lhsT = gathered Xv [128 kpos, 65] with a ones column: PSUM row
    64 accumulates the softmax denominator for free during the V streams.
  - Softmax DIVISION IS DONE ON HOST: the device DMAs out raw [65, 512]
    (d + denominator, qpos) tiles; the host divides and scatter-adds the
    three config grids. This removes the reciprocal/broadcast/multiply/add
    chains (~40us of DVE+GpSimd) and their pipeline dependencies entirely.
  - The v-projection path (vc DMA, v proj, PE gather-transposes, Xv copies)
    is emitted AFTER each chunk's attention work: it is off the critical
    scores->exp path and fills PE/DVE gaps.

key_padding_mask is all zeros by construction (spec fill=zeros) and is
therefore not applied on device.
"""

import os

# The axon-tunneled NeuronCores occasionally come up wedged from a prior
# session (NRT_EXEC_UNIT_UNRECOVERABLE on first use); requesting a core
# reset at runtime-init clears it and is a no-op on a healthy device.
os.environ.setdefault("NEURON_RT_RESET_CORES", "1")

import numpy as np

import concourse.bass as bass
import concourse.bacc as bacc
import concourse.tile as tile
from concourse import mybir
from concourse.bass_utils import run_bass_kernel_spmd

S = 4096
E = 1024
HD = 128  # head dims per core (2 heads x 64)
NCORES = 8
CHUNK = 512  # positions per projection chunk
NCHUNK = S // CHUNK
CONFIGS = [(1024, 1), (2048, 2), (4096, 4)]

# Units (ci, j) ordered by the chunk after which they are fully projected.
UNITS = [(0, 0), (0, 1), (1, 0), (0, 2), (0, 3), (1, 1), (2, 0)]
NUNITS = len(UNITS)
UID = {u: i for i, u in enumerate(UNITS)}


def _units_ready_after_chunk():
    ready = {c: [] for c in range(NCHUNK)}
    for ci, (seg, dil) in enumerate(CONFIGS):
        for j in range(S // seg):
            last_pos = (j + 1) * seg - 1
            ready[last_pos // CHUNK].append((ci, j))
    return ready


def build_bass(loop_n=None, stage_level=4):
    """loop_n: if set, wrap the whole body in an on-device For_i repeat
    loop (timing mode: marginal wall time per extra iteration = HW exec
    time, independent of host dispatch overhead)."""
    f32 = mybir.dt.float32
    bf16 = mybir.dt.bfloat16
    nc = bacc.Bacc("TRN2", target_bir_lowering=False, debug=False,
                   num_devices=NCORES)
    qT = nc.declare_dram_parameter("qT", [E, S], bf16, isOutput=False)
    kT = nc.declare_dram_parameter("kT", [E, S], bf16, isOutput=False)
    vT = nc.declare_dram_parameter("vT", [E, S], bf16, isOutput=False)
    wqT = nc.declare_dram_parameter("wqT", [E, HD], bf16, isOutput=False)
    ident = nc.declare_dram_parameter("ident", [128, 128], bf16,
                                      isOutput=False)
    # raw output tiles: [d+den, unit*2048 + h*1024 + qt*512 + qcol]
    o_out = nc.declare_dram_parameter("o_out", [65, NUNITS * 2048], f32,
                                      isOutput=True)

    ET = E // 128  # 8 E-tiles

    with tile.TileContext(nc) as tc:
        # ---- persistent SBUF tensors ----
        _frees = []

        def ptile(shape, name, dt=f32):
            t, free = tc.tile(shape, dt, name=name)
            _frees.append(free)
            return t

        wq_sb = ptile([128, ET, HD], "wq_sb", bf16)
        # X buffers ping-ponged across For_i iterations so iteration i+1's
        # DMA+projection can overlap iteration i's attention tail.
        nv_tiles = [S // 128 // dil for (seg, dil) in CONFIGS]  # 32,16,8
        XqT, XkT, Xv = [], [], []
        for pp in range(2):
            XqT.append(ptile([HD, S], f"XqT{pp}", bf16))
            XkT.append(ptile([HD, S], f"XkT{pp}", bf16))
            # Xv per config: gathered [kpos, (64|1)*2] tiles, 130 cols/tile
            Xv.append([ptile([128, n * 130], f"Xv{pp}_{i}", bf16)
                       for i, n in enumerate(nv_tiles)])
            for xv in Xv[pp]:
                nc.vector.memset(xv[:, 64::65], 1.0)  # ones cols (denom)
        id_sb = ptile([128, 128], "id_sb", bf16)

        # ---- pools ----
        import contextlib
        ctx = contextlib.ExitStack()
        with ctx:
            stage = ctx.enter_context(tc.tile_pool(name="stage", bufs=10))
            xvt_pool = ctx.enter_context(tc.tile_pool(name="xvt", bufs=3))
            wt_pool = ctx.enter_context(tc.tile_pool(name="wt", bufs=24))
            sg_pool = ctx.enter_context(tc.tile_pool(name="sg", bufs=4))
            # PSUM budget (8 banks): scores 2x[128,1024]f32 (4 banks),
            # V accumulators 2x[65,512] (2 banks), proj/transpose 2 banks.
            ps_sc = ctx.enter_context(
                tc.tile_pool(name="ps_sc", bufs=2, space="PSUM"))
            ps_v = ctx.enter_context(
                tc.tile_pool(name="ps_v", bufs=2, space="PSUM"))
            ps_wk = ctx.enter_context(
                tc.tile_pool(name="ps_wk", bufs=2, space="PSUM"))

            # load wqT: [E, HD] -> [128, ET, HD]  (scalar ring: keeps the
            # sync ring free for the first q-chunk DMA)
            nc.scalar.dma_start(
                wq_sb[:], wqT.rearrange("(a p) m -> p a m", p=128))
            nc.scalar.dma_start(id_sb[:], ident[:])

            ready = _units_ready_after_chunk()

            import contextlib as _ctxlib

            @_ctxlib.contextmanager
            def lowprio(off=100000):
                """Emit instructions with scheduler priority pushed towards
                the end of the program (deps still enforce correctness)."""
                tc.cur_priority += off
                try:
                    yield
                finally:
                    tc.cur_priority -= off

            def dma_chunk(src, nm, c, eng):
                lo = c * CHUNK
                t = stage.tile([128, ET, CHUNK], bf16, name=nm, tag="stage")
                eng.dma_start(
                    t[:],
                    src.rearrange("(a p) n -> p a n", p=128)
                       [:, :, lo:lo + CHUNK])
                return t

            def proj_x(pp, c, st, dst):
                """Project one staged chunk tensor into dst[:, chunk]."""
                lo = c * CHUNK
                ps = ps_wk.tile([128, CHUNK], f32, name="ps_proj", tag="wk")
                for e in range(ET):
                    nc.tensor.matmul(ps[:], wq_sb[:, e, :], st[:, e, :],
                                     start=(e == 0), stop=(e == ET - 1))
                nc.vector.tensor_copy(dst[:, lo:lo + CHUNK], ps[:])

            def proj_qk(pp, c):
                """q/k chunk DMA + projection: the critical path feeding
                scores->exp; emitted first within each chunk block."""
                qc = dma_chunk(qT, "qc", c, nc.sync)
                kc = dma_chunk(kT, "kc", c, nc.gpsimd)
                proj_x(pp, c, qc, XqT[pp])
                proj_x(pp, c, kc, XkT[pp])

            def proj_v(pp, c):
                """v chunk DMA + projection + gather-transposes; off the
                critical path (consumed by V matmuls ~15us later); its DMA
                is emitted after the prefetched q/k loads on the rings."""
                lo = c * CHUNK
                vc = stage.tile([128, ET, CHUNK], bf16, name="vc",
                                tag="stage")
                eng = nc.gpsimd if c % 2 == 1 else nc.sync
                eng.dma_start(
                    vc[:],
                    vT.rearrange("(a p) n -> p a n", p=128)
                      [:, :, lo:lo + CHUNK])
                ps = ps_wk.tile([128, CHUNK], f32, name="ps_vt", tag="wk")
                for e in range(ET):
                    nc.tensor.matmul(ps[:], wq_sb[:, e, :], vc[:, e, :],
                                     start=(e == 0), stop=(e == ET - 1))
                xvt = xvt_pool.tile([128, CHUNK], bf16, name="xvt", tag="xvt")
                nc.vector.tensor_copy(xvt[:], ps[:])
                for ci, (seg, dil) in enumerate(CONFIGS):
                    npt = CHUNK // dil // 128  # transposes: 4,2,1
                    for t in range(npt):
                        g = c * npt + t  # global gathered tile index
                        pt_ = ps_wk.tile([128, 128], bf16, name="ps_tr",
                                         tag="wk")
                        sl = slice(t * 128 * dil, (t + 1) * 128 * dil, dil)
                        nc.tensor.transpose(pt_[:], xvt[:, sl], id_sb[:])
                        dst = Xv[pp][ci][:, 130 * g:130 * (g + 1)] \
                            .rearrange("p (a b) -> p a b", b=65)[:, :, 0:64]
                        nc.vector.tensor_copy(
                            dst, pt_[:].rearrange("p (a b) -> p a b", b=64))

            def attention_scores(pp, ci, j, kts=range(8), wts=None,
                                 cb=None):
                """Scores + exp for one unit; returns per-(head, kt) exp'd
                weight tiles. Head A/B matmuls interleave on PE row halves.
                cb(kt) (if given) emits interleaved work after each kt
                bundle — used to pace the previous unit's V matmuls so they
                never stuff the PE exec queue ahead of scores."""
                seg, dil = CONFIGS[ci]
                r = seg // dil
                assert r == 1024
                if wts is None:
                    wts = [[None] * 8, [None] * 8]  # per head
                for kt in kts:
                    ksl = slice(j * seg + kt * 128 * dil,
                                j * seg + (kt + 1) * 128 * dil, dil)
                    pss = [ps_sc.tile([128, r], f32, name=f"ps_s{h}",
                                      tag="sc") for h in (0, 1)]
                    for half in (0, 1):
                        q2 = slice(j * seg + half * 512 * dil,
                                   j * seg + (half + 1) * 512 * dil, dil)
                        for h in (0, 1):  # interleave A/B for row packing
                            hsl = slice(64 * h, 64 * h + 64)
                            nc.tensor.matmul(
                                pss[h][:, half * 512:(half + 1) * 512],
                                XkT[pp][hsl, ksl], XqT[pp][hsl, q2])
                    if stage_level < 2:
                        continue
                    for h in (0, 1):
                        wt = wt_pool.tile([128, r], bf16, name="wt", tag="wt")
                        nc.scalar.activation(
                            wt[:], pss[h][:],
                            mybir.ActivationFunctionType.Exp, scale=1.125)
                        wts[h][kt] = wt
                    if cb is not None:
                        cb(kt)
                return wts

            def v_piece(pp, uid, ci, j, wts, ov, piece):
                """One of 8 V-burst pieces for a unit: piece p covers
                head p//4, q-half (p//2)%2, kc quad (p%2)*4..+3 — exactly
                one [65, 512] psum accumulator (d rows + denominator row
                64) is live at a time; after its kc7 matmul it is staged
                to SBUF and DMA'd out raw — softmax division happens on
                the host."""
                if stage_level < 3:
                    return
                seg, dil = CONFIGS[ci]
                gbase = j * seg // dil // 128  # Xv tile base (8 per unit)
                h, qt, kh = piece // 4, (piece // 2) % 2, piece % 2
                for kc in range(4 * kh, 4 * kh + 4):
                    lhs = Xv[pp][ci][:, 130 * (gbase + kc) + 65 * h:
                                     130 * (gbase + kc) + 65 * h + 65]
                    if kc == 0:
                        ov[h][qt] = ps_v.tile([65, 512], f32,
                                              name="ov", tag="ov")
                    nc.tensor.matmul(
                        ov[h][qt][:], lhs,
                        wts[h][kc][:, qt * 512:(qt + 1) * 512],
                        start=(kc == 0), stop=(kc == 7))
                if kh == 1 and stage_level >= 4:
                    sg = sg_pool.tile([65, 512], f32, name="sg", tag="sg")
                    nc.vector.tensor_copy(sg[:], ov[h][qt][:])
                    base = uid * 2048 + h * 1024 + qt * 512
                    eng = nc.sync if (h + qt) % 2 == 0 else nc.gpsimd
                    eng.dma_start(o_out[:, base:base + 512], sg[:])

            def attention_v(pp, uid, ci, j, wts):
                """All 8 V pieces back to back (used for the final unit;
                its matmuls still dispatch early, piece by piece, as the
                exp tiles they need are produced)."""
                ov = [[None, None], [None, None]]
                for piece in range(8):
                    v_piece(pp, uid, ci, j, wts, ov, piece)

            def mk_vcb(pp, pv):
                if pv is None:
                    return None
                ov = [[None, None], [None, None]]

                def cb(kt):
                    v_piece(pp, *pv, ov, kt)
                return cb

            def body(pp):
                # Emission (= scheduler priority) order per chunk:
                # q/k DMA+proj, then all newly-ready units' scores+exp,
                # then the v-projection path, then pending V bursts (the
                # V deps land after proj_v so program-order deps hold;
                # the list scheduler hoists ready V matmuls into PE gaps
                # while ACT paces through the exp queue).
                pending = None  # (uid, ci, j, wts)
                # Fast lead-in: q/k DMAs for chunks 0-3 go out up front on
                # 3 rings (DMA queues drain in emission order; prefetching
                # keeps scores-feeding loads ahead of vc/o_out traffic),
                # and unit (0,0)'s scores kt0-3 (which need only q0/q1/k0)
                # are emitted before the k1 projection so ACT starts early.
                qk = {}
                qk[0] = (dma_chunk(qT, "qc", 0, nc.sync),
                         dma_chunk(kT, "kc", 0, nc.gpsimd))
                qk[1] = (dma_chunk(qT, "qc", 1, nc.scalar),
                         dma_chunk(kT, "kc", 1, nc.sync))
                qk[2] = (dma_chunk(qT, "qc", 2, nc.gpsimd),
                         dma_chunk(kT, "kc", 2, nc.sync))
                qk[3] = (dma_chunk(qT, "qc", 3, nc.gpsimd),
                         dma_chunk(kT, "kc", 3, nc.sync))
                proj_x(pp, 0, qk[0][0], XqT[pp])
                proj_x(pp, 0, qk[0][1], XkT[pp])
                proj_x(pp, 1, qk[1][0], XqT[pp])
                if stage_level >= 1:
                    wts00 = attention_scores(pp, 0, 0, kts=range(0, 4))
                proj_x(pp, 1, qk[1][1], XkT[pp])
                if stage_level >= 1:
                    attention_scores(pp, 0, 0, kts=range(4, 8), wts=wts00)
                    pending = (UID[(0, 0)], 0, 0, wts00)
                proj_v(pp, 0)
                proj_v(pp, 1)
                pend = [(pending, 1)] if pending is not None else []
                for c in range(2, NCHUNK):
                    if c + 2 < NCHUNK:  # depth-2 q/k prefetch
                        qk[c + 2] = (dma_chunk(qT, "qc", c + 2, nc.gpsimd),
                                     dma_chunk(kT, "kc", c + 2, nc.sync))
                    proj_x(pp, c, qk[c][0], XqT[pp])
                    proj_x(pp, c, qk[c][1], XkT[pp])
                    del qk[c]
                    if stage_level >= 1:
                        for (ci, j) in ready[c]:
                            # interleave the oldest pending unit's V pieces
                            # into this unit's scores — legal only if its
                            # v-projection (chunk < c) is already emitted
                            cb = None
                            if pend and pend[0][1] < c:
                                cb = mk_vcb(pp, pend.pop(0)[0])
                            wts = attention_scores(pp, ci, j, cb=cb)
                            pend.append(((UID[(ci, j)], ci, j, wts), c))
                    proj_v(pp, c)
                for args, _ in pend:
                    attention_v(pp, *args)

            if loop_n is None:
                body(0)
            else:
                assert loop_n % 2 == 0, "loop_n must be even (ping-pong)"
                with tc.For_i(0, loop_n // 2, 1):
                    body(0)
                    body(1)

        for f in reversed(_frees):
            f()

    nc.compile()
    return nc


_CACHED = {}


def make_in_maps(query, key, value, Wq):
    """Host-side input prep: bf16 transposes + the Wq/3 scaling trick."""
    import ml_dtypes
    bf = ml_dtypes.bfloat16
    qT = np.ascontiguousarray(query[0].T).astype(bf)
    kT = np.ascontiguousarray(key[0].T).astype(bf)
    vT = np.ascontiguousarray(value[0].T).astype(bf)
    ident = np.eye(128, dtype=np.float32).astype(bf)
    in_maps = []
    for c in range(NCORES):
        wqTc = np.ascontiguousarray(
            (Wq[HD * c:HD * (c + 1), :] / 3.0).T).astype(bf)
        in_maps.append({"qT": qT, "kT": kT, "vT": vT, "wqT": wqTc,
                        "ident": ident})
    return in_maps


def assemble(o_outs):
    """Host-side softmax division + strided scatter-sum of the 3 config
    grids. o_outs: per-core [65, NUNITS*2048] f32 raw tiles."""
    out = np.zeros((S, E), np.float32)
    for c, o in enumerate(o_outs):
        t = np.asarray(o, np.float32).reshape(65, NUNITS, 2, 2, 512)
        val = t[0:64] / t[64:65]          # [64, u, h, qt, col]
        for ci, (seg, dil) in enumerate(CONFIGS):
            n_seg = S // seg
            uids = [UID[(ci, j)] for j in range(n_seg)]
            v = val[:, uids]              # [64, n_seg, h, qt, col]
            # gathered pos = qt*512+col ; head dims = h*64+d
            g = v.transpose(1, 3, 4, 2, 0).reshape(n_seg, 1024, HD)
            out.reshape(n_seg, seg, E)[:, ::dil, HD * c:HD * (c + 1)] += g
    return out


def kernel(query, key, value, key_padding_mask, Wq):
    query = np.asarray(query, dtype=np.float32)
    key = np.asarray(key, dtype=np.float32)
    value = np.asarray(value, dtype=np.float32)
    Wq = np.asarray(Wq, dtype=np.float32)
    assert query.shape == (1, S, E), query.shape

    if "nc" not in _CACHED:
        _CACHED["nc"] = build_bass()
    nc = _CACHED["nc"]

    in_maps = make_in_maps(query, key, value, Wq)
    res = run_bass_kernel_spmd(nc, in_maps, list(range(NCORES)))
    out = assemble([res.results[c]["o_out"] for c in range(NCORES)])
    return out[None].astype(np.float32)
